# revision 33
# baseline (speedup 1.0000x reference)
"""Trainium2 Bass kernel for nn_AttentionHead_5583457485447 (sparse_attention).

Reference computation (per batch b):
    q = X @ Wq; k = X @ Wk                      # [N, DK]
    s = relu((q @ k.T) / sqrt(DK)) * M_mask     # [N, N]
    out = s @ Z @ Wv                            # [N, DV]

Strategy (8 NeuronCores, data-parallel over batch B=8, one batch per core):
  - Fold 1/sqrt(DK) into Wv; fold Wv into Z on device (ZW = Z @ Wv/8) so the
    masked-score matrix feeds one big matmul: out = maskedT.T @ ZW.
  - Scores computed directly in transposed [m, n] layout (lhsT = kT,
    rhs = qT) -- already the lhsT layout the C matmul needs.
  - Combined projection [Wq|Wk] / [Wk|Qq] gives qkT/kqT tiles whose row
    halves let the two score matmuls of an (even, odd) m-tile pair run in
    PE row groups 0-63 / 64-127 concurrently (tile_position auto-derive).
  - m-major streaming: mask rows are DMAed per 128-row block; scores +
    relu*mask follow each block.  The C matmul runs in two m-half passes
    (K-contiguous 8-matmul PSUM chains per n-tile): half-A overlaps the
    mask DMA + scores of rows 8..15, half-B adds the spilled partial.
  - relu + mask rotated across engines (DVE fused max*mult, or ACT relu +
    DVE/GpSimd bf16 multiply); all matmul inputs bf16, accumulation fp32.
"""

import json
import os
import sys

import numpy as np

B, N, D, DK = 8, 2048, 256, 64
DV = D + 1  # 257
NT = N // 128  # 16 tiles along n and along m
PW = 512  # scores matmul moving width
SW = 1024  # fused relu-mask op width (psum tile width, 2 banks)

LAST_EXEC_NS = None
_CACHE = {}


# --------------------------------------------------------------------------
# Patch 1: this container's walrus build rejects instructions carrying more
# than one semaphore wait. Split excess waits onto same-engine NOPs at the
# serialized-BIR level (generic, covers Tile's drains and compute ops).
# --------------------------------------------------------------------------
def _split_waits_in_bir(bir_json: bytes) -> bytes:
    bir = json.loads(bir_json)
    changed = False
    drop_ldw = os.environ.get("KERNEL_DROP_LDW", "0") == "1"
    for fn in bir.get("functions", []):
        for bb in fn.get("blocks", []):
            insts = bb.get("instructions", [])
            if drop_ldw:
                # Remove standalone Ldweights prefetches (the paired Matmult is
                # self-loading: it carries both operands). Merge their sync
                # info into the following Matmult on the same engine.
                merged = []
                pend = {}
                for inst in insts:
                    if inst.get("opcode") == "Ldweights":
                        si = inst.get("sync_info") or {}
                        if si.get("on_wait") or si.get("on_update"):
                            pend.setdefault(inst["engine"], []).append(si)
                        changed = True
                        continue
                    if inst.get("opcode") == "Matmult" and pend.get(inst.get("engine")):
                        tgt = inst.setdefault("sync_info", {"on_update": [], "on_wait": []})
                        tgt.setdefault("on_wait", [])
                        tgt.setdefault("on_update", [])
                        for si in pend.pop(inst["engine"]):
                            tgt["on_wait"] += si.get("on_wait") or []
                            tgt["on_update"] += si.get("on_update") or []
                    merged.append(inst)
                insts = merged
            out = []
            for inst in insts:
                si = inst.get("sync_info")
                ow = (si or {}).get("on_wait") or []
                if len(ow) > 1:
                    changed = True
                    for i, w in enumerate(ow[:-1]):
                        out.append({
                            "debug": inst.get("debug", 0),
                            "engine": inst["engine"],
                            "ins": [],
                            "name": f"{inst['name']}-ws{i}",
                            "opcode": "NoOp",
                            "outs": [],
                            "sync_info": {"on_update": [], "on_wait": [w]},
                            "text_hint": "wait_split",
                        })
                    si["on_wait"] = [ow[-1]]
                out.append(inst)
            bb["instructions"] = out
    return json.dumps(bir).encode() if changed else bir_json


def _apply_bir_patch():
    import concourse.bass_utils as bass_utils
    import concourse.bass2jax as bass2jax

    if os.environ.get("KERNEL_LDW_OPT", "0") == "1":
        rc_orig = bass_utils.run_command
        if not getattr(rc_orig, "_ldw_wrapped", False):
            def rc_wrapped(argv, **kwargs):
                argv = [a.replace("--enable-ldw-opt=false", "--enable-ldw-opt=true")
                        if isinstance(a, str) else a for a in argv]
                return rc_orig(argv, **kwargs)
            rc_wrapped._ldw_wrapped = True
            bass_utils.run_command = rc_wrapped

    orig = bass_utils.compile_bir_kernel
    if getattr(orig, "_wait_split_wrapped", False):
        return

    def wrapped(bir_json, tmpdir, neff_name="file.neff"):
        if isinstance(bir_json, str):
            bir_json = bir_json.encode()
        return orig(_split_waits_in_bir(bir_json), tmpdir, neff_name=neff_name)

    wrapped._wait_split_wrapped = True
    bass_utils.compile_bir_kernel = wrapped
    bass2jax.compile_bir_kernel = wrapped


# --------------------------------------------------------------------------
# Patch 2: optional NTFF profiling hook for axon (exec-time measurement).
# Only used when KERNEL_TRACE=1; missing in this image's antenv.
# --------------------------------------------------------------------------
def _install_profile_shim():
    import types, ctypes, contextlib

    if "antenv.axon_hooks" in sys.modules:
        return
    so_path = "/opt/axon/libaxon_pjrt.so"
    if not os.path.exists(so_path):
        return
    lib = ctypes.CDLL(so_path)
    if not hasattr(lib, "axon_start_nrt_profile"):
        return
    lib.axon_start_nrt_profile.argtypes = [ctypes.POINTER(ctypes.c_int64), ctypes.c_size_t]
    lib.axon_start_nrt_profile.restype = ctypes.c_int64
    lib.axon_stop_nrt_profile.argtypes = [ctypes.c_char_p]
    lib.axon_stop_nrt_profile.restype = ctypes.c_int64

    @contextlib.contextmanager
    def _hook(output_dir, device_ids):
        import jax

        jax.devices()
        if device_ids:
            ids = (ctypes.c_int64 * len(device_ids))(*device_ids)
            rc = lib.axon_start_nrt_profile(ids, len(device_ids))
        else:
            rc = lib.axon_start_nrt_profile(None, 0)
        if rc != 0:
            raise RuntimeError(f"axon_start_nrt_profile rc={rc}")
        try:
            yield
        finally:
            n = lib.axon_stop_nrt_profile(str(output_dir).encode())
            print(f"profile: {n} file(s) written to {output_dir}", file=sys.stderr)

    mod = types.ModuleType("antenv.axon_hooks")
    mod.get_axon_ntff_profile_hook = lambda: _hook
    sys.modules["antenv.axon_hooks"] = mod


# --------------------------------------------------------------------------
# Device program (identical for all 8 cores; one batch per core)
# --------------------------------------------------------------------------
def _build_nc():
    import concourse.bass as bass
    import concourse.mybir as mybir
    import concourse.tile as tile

    f32 = mybir.dt.float32
    bf16 = mybir.dt.bfloat16
    Alu = mybir.AluOpType
    Act = mybir.ActivationFunctionType

    nc = bass.Bass("TRN2", debug=False)

    d_maskT = nc.dram_tensor("maskT", [N, N], bf16, kind="ExternalInput")
    d_XT = nc.dram_tensor("XT", [D, N], bf16, kind="ExternalInput")
    d_ZT = nc.dram_tensor("ZT", [DV, N], bf16, kind="ExternalInput")
    d_Wqk = nc.dram_tensor("Wqk", [D, 128], bf16, kind="ExternalInput")
    d_Wkq = nc.dram_tensor("Wkq", [D, 128], bf16, kind="ExternalInput")
    d_Wv8 = nc.dram_tensor("Wv8", [DV, DV], bf16, kind="ExternalInput")
    # out[p, nt*DV + v] = result[nt*128 + p, v]; host de-interleaves.  The
    # [128, NT*DV] layout gives multi-KB DMA descriptors per partition row.
    d_out = nc.dram_tensor("out", [128, NT * DV], f32, kind="ExternalOutput")

    SW = 1024  # elementwise unit width (2 PSUM banks)

    with tile.TileContext(nc) as tc:
        with (
            tc.tile_pool(name="wts", bufs=1) as wts,          # weights/XT/ZT/qkT/zw
            tc.tile_pool(name="maskp", bufs=16) as maskp,     # maskT stream [128, SW]
            tc.tile_pool(name="mskd", bufs=NT) as mskdp,      # persistent masked tiles
            tc.tile_pool(name="pAp", bufs=NT) as pAp,         # half-A C partials (f32)
            tc.tile_pool(name="rlp", bufs=6) as rlp,          # relu staging (ACT path)
            tc.tile_pool(name="outp", bufs=4) as outp,        # out staging
            tc.tile_pool(name="psS", bufs=4, space="PSUM") as psS,  # 4 x 2 banks
        ):
            # ---- PE warm-up: dummy matmuls engage the HAM clock un-throttle
            # (K=8/8, 2.4 GHz) while the first DMAs stream in. ----
            wu = wts.tile([128, PW], bf16, tag="wu", name="wu")
            nc.vector.memset(wu[:], 0.0)
            for w in range(10):
                pw = psS.tile([128, SW], f32, tag="psS", name=f"psw{w}")
                nc.tensor.matmul(pw[:, :PW], wu[:, :128], wu[:], start=True, stop=True)

            # ---- input DMAs, spread over queues so X/Z/W get an early
            # bandwidth share against the mask stream (on sync) ----
            xt_sb = [wts.tile([128, N], bf16, tag=f"xt{c}", name=f"xt{c}") for c in range(2)]
            nc.scalar.dma_start(xt_sb[0][:], d_XT.ap()[0:128, :])
            nc.gpsimd.dma_start(xt_sb[1][:], d_XT.ap()[128:256, :])
            wqk_sb = [wts.tile([128, 128], bf16, tag=f"wqk{c}", name=f"wqk{c}") for c in range(2)]
            wkq_sb = [wts.tile([128, 128], bf16, tag=f"wkq{c}", name=f"wkq{c}") for c in range(2)]
            for c in range(2):
                nc.scalar.dma_start(wqk_sb[c][:], d_Wqk.ap()[c * 128:(c + 1) * 128, :])
                nc.gpsimd.dma_start(wkq_sb[c][:], d_Wkq.ap()[c * 128:(c + 1) * 128, :])
            vchunks = [(0, 128), (128, 128), (256, 1)]
            wv_sb = [wts.tile([p, DV], bf16, tag=f"wv{i}", name=f"wv{i}") for i, (v0, p) in enumerate(vchunks)]
            zt_sb = [wts.tile([p, N], bf16, tag=f"zt{i}", name=f"zt{i}") for i, (v0, p) in enumerate(vchunks)]
            for i, (v0, p) in enumerate(vchunks):
                nc.gpsimd.dma_start(wv_sb[i][:], d_Wv8.ap()[v0:v0 + p, :])
                nc.scalar.dma_start(zt_sb[i][:], d_ZT.ap()[v0:v0 + p, :])

            # ---- projections: qkT = [q;k], kqT = [k;q] along partitions ----
            # qkT rows 0:64 = qT, rows 64:128 = kT (kqT swapped).  The score
            # matmul pair for (mt even, mt odd) then runs in PE row groups
            # 0-63 / 64-127 concurrently with no on-chip duplication.
            # ---- projections, two 512-col chains per 2-bank tile, one wide
            # evac each ----
            qkT = wts.tile([128, N], bf16, tag="qkT", name="qkT")
            kqT = wts.tile([128, N], bf16, tag="kqT", name="kqT")
            for si, (dst, w_sb) in enumerate(((qkT, wqk_sb), (kqT, wkq_sb))):
                for g in range(N // SW):
                    ps = psS.tile([128, SW], f32, tag="psS", name=f"psa{si}_{g}")
                    for h in range(2):
                        for c in range(2):
                            nc.tensor.matmul(
                                ps[:, h * PW:(h + 1) * PW],
                                w_sb[c][:],
                                xt_sb[c][:, g * SW + h * PW:g * SW + (h + 1) * PW],
                                start=(c == 0),
                                stop=(c == 1),
                            )
                    if g % 2 == 0:
                        nc.vector.tensor_copy(dst[:, g * SW:(g + 1) * SW], ps[:])
                    else:
                        nc.scalar.activation(dst[:, g * SW:(g + 1) * SW], ps[:], Act.Copy)

            # ---- ZW = Z @ (Wv/sqrt(dk)); two m-tiles share one 2-bank tile
            # (each 257-col chain stays inside its own bank) ----
            zw_sb = {}

            def emit_zw_pair(mt0):
                ps = psS.tile([128, SW], f32, tag="psS", name=f"pzw{mt0}")
                for j, mt in enumerate((mt0, mt0 + 1)):
                    for i in range(3):
                        nc.tensor.matmul(
                            ps[:, j * PW:j * PW + DV],
                            zt_sb[i][:, mt * 128:(mt + 1) * 128],
                            wv_sb[i][:],
                            start=(i == 0),
                            stop=(i == 2),
                        )
                for j, mt in enumerate((mt0, mt0 + 1)):
                    zw = wts.tile([128, DV], bf16, tag=f"zw{mt}", name=f"zw{mt}")
                    if mt % 2 == 0:
                        nc.scalar.activation(zw[:], ps[:, j * PW:j * PW + DV], Act.Copy)
                    else:
                        nc.vector.tensor_copy(zw[:], ps[:, j * PW:j * PW + DV])
                    zw_sb[mt] = zw

            # ---- score pair: masked[mt] = relu(sT)*maskT for mts (2pr,2pr+1) ----
            # Elementwise runs on 1024-wide units; per pair there are 4 units.
            # pattern: tuple of unit modes, "v" = DVE fused max*mult,
            # "g"/"d" = ACT relu then GpSimd/DVE bf16 multiply.
            masked_sb = {}
            warm_ps = []  # recently consumed score tiles for keep-warm matmuls

            def emit_score_pair(pr, pattern):
                mts = (2 * pr, 2 * pr + 1)
                mds, mks = [], []
                for mt in mts:
                    mk = maskp.tile([128, N], bf16, tag="mask", name=f"mk{mt}")
                    if mt == 0:
                        # transitive gate: the whole mask stream (sync queue,
                        # FIFO) starts only after X has landed, so the
                        # projection inputs get the DMA bandwidth first.
                        nc.sync.dma_start(mk[0:1, 0:2], xt_sb[0][0:1, 0:2])
                        nc.sync.dma_start(mk[0:1, 2:4], xt_sb[1][0:1, 0:2])
                    nc.sync.dma_start(mk[:], d_maskT.ap()[mt * 128:(mt + 1) * 128, :])
                    mks.append(mk)
                    md = mskdp.tile([128, N], bf16, tag="masked", name=f"md{mt}")
                    mds.append(md)
                    masked_sb[mt] = md
                for u in range(N // SW):
                    pss = []
                    for j, mt in enumerate(mts):
                        ro = DK * j
                        lhs_t = (kqT, qkT)[j]  # rows ro:ro+64 hold kT
                        rhs_t = (qkT, kqT)[j]  # rows ro:ro+64 hold qT
                        ps = psS.tile([128, SW], f32, tag="psS", name=f"pss{mt}_{u}")
                        for h in range(SW // PW):
                            c0 = u * SW + h * PW
                            nc.tensor.matmul(
                                ps[:, h * PW:(h + 1) * PW],
                                lhs_t[ro:ro + DK, mt * 128:(mt + 1) * 128],
                                rhs_t[ro:ro + DK, c0:c0 + PW],
                                start=True,
                                stop=True,
                            )
                        pss.append(ps)
                    for j, mt in enumerate(mts):
                        sl = slice(u * SW, (u + 1) * SW)
                        mode = pattern[2 * u + j]
                        if mode == "v":
                            nc.vector.scalar_tensor_tensor(
                                mds[j][:, sl], pss[j][:], 0.0, mks[j][:, sl],
                                Alu.max, Alu.mult,
                            )
                        else:
                            rl = rlp.tile([128, SW], bf16, tag="rl", name=f"rl{mt}_{u}")
                            nc.scalar.activation(rl[:], pss[j][:], Act.Relu)
                            eng = nc.gpsimd if mode == "g" else nc.vector
                            eng.tensor_mul(mds[j][:, sl], rl[:], mks[j][:, sl])
                        warm_ps.append(pss[j])

            def emit_c_pair(nt0, mt_range):
                # two 16-matmul full-contraction chains (nt0, nt0+1) into the
                # two banks of one 2-bank tile, evac straight to the out tile
                ps = psS.tile([128, SW], f32, tag="psS", name=f"psc{nt0}_{mt_range[0]}")
                for j, nt in enumerate((nt0, nt0 + 1)):
                    for i, mt in enumerate(mt_range):
                        nc.tensor.matmul(
                            ps[:, j * PW:j * PW + DV],
                            masked_sb[mt][:, nt * 128:(nt + 1) * 128],
                            zw_sb[mt][:],
                            start=(i == 0),
                            stop=(i == len(mt_range) - 1),
                        )
                ot = outp.tile([128, 2 * DV], f32, tag="out", name=f"ot{nt0}")
                for j, nt in enumerate((nt0, nt0 + 1)):
                    psl = ps[:, j * PW:j * PW + DV]
                    if nt % 2 == 0:
                        nc.scalar.activation(ot[:, j * DV:(j + 1) * DV], psl, Act.Copy)
                    else:
                        nc.vector.tensor_copy(ot[:, j * DV:(j + 1) * DV], psl)
                nc.sync.dma_start(
                    d_out.ap()[:, nt0 * DV:(nt0 + 2) * DV], ot[:]
                )

            # phase A: zw first (dedicated early slice of the psS ring), then
            # scores for m rows 0..1023 (mask DMA-paced).
            PAT_A = ("v", "g", "v", "d")
            PAT_B = ("v", "d", "v", "g")
            PAT_TAIL = ("v", "d", "v", "d")  # lowest-latency path for last pairs
            for mt0 in range(0, NT, 2):
                emit_zw_pair(mt0)
            for pr in range(4):
                emit_score_pair(pr, PAT_A)

            # remaining score pairs (m rows 1024..2047)
            for pr in range(4, 8):
                emit_score_pair(pr, PAT_TAIL if pr >= 6 else PAT_B)

            # phase C: one full-contraction pass (no partial spills); the
            # first chain starts streaming as its masked rows land.
            for k in range(NT // 2):
                emit_c_pair(2 * k, list(range(NT)))

    return nc


def kernel(Z_l, X_l, M_mask, Wq, Wk, Wv):
    global LAST_EXEC_NS
    _apply_bir_patch()

    trace = os.environ.get("KERNEL_TRACE", "0") == "1"
    if trace:
        _install_profile_shim()

    from concourse.bass_utils import run_bass_kernel_spmd

    Z_l = np.asarray(Z_l, dtype=np.float32)
    X_l = np.asarray(X_l, dtype=np.float32)
    M_mask = np.asarray(M_mask, dtype=np.float32)
    Wq = np.asarray(Wq, dtype=np.float32)
    Wk = np.asarray(Wk, dtype=np.float32)
    Wv = np.asarray(Wv, dtype=np.float32)

    import ml_dtypes
    bf = ml_dtypes.bfloat16

    # Host-side layout prep (transpose + bf16 cast) + scale fold.
    XT = np.ascontiguousarray(X_l.transpose(0, 2, 1)).astype(bf)     # [B, D, N]
    ZT = np.ascontiguousarray(Z_l.transpose(0, 2, 1)).astype(bf)     # [B, DV, N]
    MT = np.ascontiguousarray(M_mask.transpose(0, 2, 1)).astype(bf)  # [B, N(m), N(n)]
    Wv8 = (Wv / np.sqrt(np.float32(DK))).astype(bf)
    Wqk = np.ascontiguousarray(np.concatenate([Wq, Wk], axis=1)).astype(bf)  # [D, 128]
    Wkq = np.ascontiguousarray(np.concatenate([Wk, Wq], axis=1)).astype(bf)  # [D, 128]

    if "nc" not in _CACHE:
        _CACHE["nc"] = _build_nc()
    nc = _CACHE["nc"]

    in_maps = [
        {
            "maskT": MT[b],
            "XT": XT[b],
            "ZT": ZT[b],
            "Wqk": Wqk,
            "Wkq": Wkq,
            "Wv8": Wv8,
        }
        for b in range(B)
    ]
    try:
        res = run_bass_kernel_spmd(nc, in_maps, core_ids=list(range(B)), trace=trace)
    except Exception:
        # A prior (profiled) run can leave an execution unit wedged; the failed
        # attempt clears it and a retry goes through.
        res = run_bass_kernel_spmd(nc, in_maps, core_ids=list(range(B)), trace=trace)
    _CACHE["last_res"] = res
    if trace:
        LAST_EXEC_NS = res.exec_time_ns
    out = np.stack(
        [
            res.results[b]["out"].reshape(128, NT, DV).transpose(1, 0, 2).reshape(N, DV)
            for b in range(B)
        ],
        axis=0,
    )
    return out



# revision 35
# speedup vs baseline: 1.1933x; 1.1933x over previous
"""Trainium2 Bass kernel for nn_AttentionHead_5583457485447 (sparse_attention).

Reference computation (per batch b):
    q = X @ Wq; k = X @ Wk                      # [N, DK]
    s = relu((q @ k.T) / sqrt(DK)) * M_mask     # [N, N]
    out = s @ Z @ Wv                            # [N, DV]

Strategy (8 NeuronCores, data-parallel over batch B=8, one batch per core):
  - Fold 1/sqrt(DK) into Wv; fold Wv into Z on device (ZW = Z @ Wv/8) so the
    masked-score matrix feeds one big matmul: out = maskedT.T @ ZW.
  - Scores computed directly in transposed [m, n] layout (lhsT = kT,
    rhs = qT) -- already the lhsT layout the C matmul needs.
  - Combined projection [Wq|Wk] / [Wk|Qq] gives qkT/kqT tiles whose row
    halves let the two score matmuls of an (even, odd) m-tile pair run in
    PE row groups 0-63 / 64-127 concurrently (tile_position auto-derive).
  - m-major streaming: mask rows are DMAed per 128-row block; scores +
    relu*mask follow each block.  The C matmul runs in two m-half passes
    (K-contiguous 8-matmul PSUM chains per n-tile): half-A overlaps the
    mask DMA + scores of rows 8..15, half-B adds the spilled partial.
  - relu + mask rotated across engines (DVE fused max*mult, or ACT relu +
    DVE/GpSimd bf16 multiply); all matmul inputs bf16, accumulation fp32.
"""

import json
import os
import sys

import numpy as np

B, N, D, DK = 8, 2048, 256, 64
DV = D + 1  # 257
NT = N // 128  # 16 tiles along n and along m
PW = 512  # scores matmul moving width
SW = 1024  # fused relu-mask op width (psum tile width, 2 banks)

LAST_EXEC_NS = None
_CACHE = {}


# --------------------------------------------------------------------------
# Patch 1: this container's walrus build rejects instructions carrying more
# than one semaphore wait. Split excess waits onto same-engine NOPs at the
# serialized-BIR level (generic, covers Tile's drains and compute ops).
# --------------------------------------------------------------------------
def _split_waits_in_bir(bir_json: bytes) -> bytes:
    bir = json.loads(bir_json)
    changed = False
    drop_ldw = os.environ.get("KERNEL_DROP_LDW", "0") == "1"
    for fn in bir.get("functions", []):
        for bb in fn.get("blocks", []):
            insts = bb.get("instructions", [])
            if drop_ldw:
                # Remove standalone Ldweights prefetches (the paired Matmult is
                # self-loading: it carries both operands). Merge their sync
                # info into the following Matmult on the same engine.
                merged = []
                pend = {}
                for inst in insts:
                    if inst.get("opcode") == "Ldweights":
                        si = inst.get("sync_info") or {}
                        if si.get("on_wait") or si.get("on_update"):
                            pend.setdefault(inst["engine"], []).append(si)
                        changed = True
                        continue
                    if inst.get("opcode") == "Matmult" and pend.get(inst.get("engine")):
                        tgt = inst.setdefault("sync_info", {"on_update": [], "on_wait": []})
                        tgt.setdefault("on_wait", [])
                        tgt.setdefault("on_update", [])
                        for si in pend.pop(inst["engine"]):
                            tgt["on_wait"] += si.get("on_wait") or []
                            tgt["on_update"] += si.get("on_update") or []
                    merged.append(inst)
                insts = merged
            out = []
            for inst in insts:
                si = inst.get("sync_info")
                ow = (si or {}).get("on_wait") or []
                if len(ow) > 1:
                    changed = True
                    for i, w in enumerate(ow[:-1]):
                        out.append({
                            "debug": inst.get("debug", 0),
                            "engine": inst["engine"],
                            "ins": [],
                            "name": f"{inst['name']}-ws{i}",
                            "opcode": "NoOp",
                            "outs": [],
                            "sync_info": {"on_update": [], "on_wait": [w]},
                            "text_hint": "wait_split",
                        })
                    si["on_wait"] = [ow[-1]]
                out.append(inst)
            bb["instructions"] = out
    return json.dumps(bir).encode() if changed else bir_json


def _apply_bir_patch():
    import concourse.bass_utils as bass_utils
    import concourse.bass2jax as bass2jax

    if os.environ.get("KERNEL_LDW_OPT", "0") == "1":
        rc_orig = bass_utils.run_command
        if not getattr(rc_orig, "_ldw_wrapped", False):
            def rc_wrapped(argv, **kwargs):
                argv = [a.replace("--enable-ldw-opt=false", "--enable-ldw-opt=true")
                        if isinstance(a, str) else a for a in argv]
                return rc_orig(argv, **kwargs)
            rc_wrapped._ldw_wrapped = True
            bass_utils.run_command = rc_wrapped

    orig = bass_utils.compile_bir_kernel
    if getattr(orig, "_wait_split_wrapped", False):
        return

    def wrapped(bir_json, tmpdir, neff_name="file.neff"):
        if isinstance(bir_json, str):
            bir_json = bir_json.encode()
        return orig(_split_waits_in_bir(bir_json), tmpdir, neff_name=neff_name)

    wrapped._wait_split_wrapped = True
    bass_utils.compile_bir_kernel = wrapped
    bass2jax.compile_bir_kernel = wrapped


# --------------------------------------------------------------------------
# Patch 2: optional NTFF profiling hook for axon (exec-time measurement).
# Only used when KERNEL_TRACE=1; missing in this image's antenv.
# --------------------------------------------------------------------------
def _install_profile_shim():
    import types, ctypes, contextlib

    if "antenv.axon_hooks" in sys.modules:
        return
    so_path = "/opt/axon/libaxon_pjrt.so"
    if not os.path.exists(so_path):
        return
    lib = ctypes.CDLL(so_path)
    if not hasattr(lib, "axon_start_nrt_profile"):
        return
    lib.axon_start_nrt_profile.argtypes = [ctypes.POINTER(ctypes.c_int64), ctypes.c_size_t]
    lib.axon_start_nrt_profile.restype = ctypes.c_int64
    lib.axon_stop_nrt_profile.argtypes = [ctypes.c_char_p]
    lib.axon_stop_nrt_profile.restype = ctypes.c_int64

    @contextlib.contextmanager
    def _hook(output_dir, device_ids):
        import jax

        jax.devices()
        if device_ids:
            ids = (ctypes.c_int64 * len(device_ids))(*device_ids)
            rc = lib.axon_start_nrt_profile(ids, len(device_ids))
        else:
            rc = lib.axon_start_nrt_profile(None, 0)
        if rc != 0:
            raise RuntimeError(f"axon_start_nrt_profile rc={rc}")
        try:
            yield
        finally:
            n = lib.axon_stop_nrt_profile(str(output_dir).encode())
            print(f"profile: {n} file(s) written to {output_dir}", file=sys.stderr)

    mod = types.ModuleType("antenv.axon_hooks")
    mod.get_axon_ntff_profile_hook = lambda: _hook
    sys.modules["antenv.axon_hooks"] = mod


# --------------------------------------------------------------------------
# Device program (identical for all 8 cores; one batch per core)
# --------------------------------------------------------------------------
def _build_nc():
    import concourse.bass as bass
    import concourse.mybir as mybir
    import concourse.tile as tile

    f32 = mybir.dt.float32
    bf16 = mybir.dt.bfloat16
    Alu = mybir.AluOpType
    Act = mybir.ActivationFunctionType

    nc = bass.Bass("TRN2", debug=False)

    d_maskT = nc.dram_tensor("maskT", [N, N], bf16, kind="ExternalInput")
    d_XT = nc.dram_tensor("XT", [D, N], bf16, kind="ExternalInput")
    d_ZT = nc.dram_tensor("ZT", [DV, N], bf16, kind="ExternalInput")
    d_Wqk = nc.dram_tensor("Wqk", [D, 128], bf16, kind="ExternalInput")
    d_Wkq = nc.dram_tensor("Wkq", [D, 128], bf16, kind="ExternalInput")
    d_Wv8 = nc.dram_tensor("Wv8", [DV, DV], bf16, kind="ExternalInput")
    # out[p, nt*DV + v] = result[nt*128 + p, v]; host de-interleaves.  The
    # [128, NT*DV] layout gives multi-KB DMA descriptors per partition row.
    d_out = nc.dram_tensor("out", [128, NT * DV], f32, kind="ExternalOutput")

    SW = 1024  # elementwise unit width (2 PSUM banks)

    with tile.TileContext(nc) as tc:
        with (
            tc.tile_pool(name="wts", bufs=1) as wts,          # weights/XT/ZT/qkT/zw
            tc.tile_pool(name="maskp", bufs=16) as maskp,     # maskT stream [128, SW]
            tc.tile_pool(name="mskd", bufs=NT) as mskdp,      # persistent masked tiles
            tc.tile_pool(name="pAp", bufs=NT) as pAp,         # half-A C partials (f32)
            tc.tile_pool(name="rlp", bufs=6) as rlp,          # relu staging (ACT path)
            tc.tile_pool(name="outp", bufs=4) as outp,        # out staging
            tc.tile_pool(name="psS", bufs=4, space="PSUM") as psS,  # 4 x 2 banks
        ):
            # ---- PE warm-up: dummy matmuls engage the HAM clock un-throttle
            # (K=8/8, 2.4 GHz) while the first DMAs stream in. ----
            wu = wts.tile([128, PW], bf16, tag="wu", name="wu")
            nc.vector.memset(wu[:], 0.0)
            for w in range(10):
                pw = psS.tile([128, SW], f32, tag="psS", name=f"psw{w}")
                nc.tensor.matmul(pw[:, :PW], wu[:, :128], wu[:], start=True, stop=True)

            # ---- input DMAs, spread over queues so X/Z/W get an early
            # bandwidth share against the mask stream (on sync) ----
            xt_sb = [wts.tile([128, N], bf16, tag=f"xt{c}", name=f"xt{c}") for c in range(2)]
            nc.scalar.dma_start(xt_sb[0][:], d_XT.ap()[0:128, :])
            nc.gpsimd.dma_start(xt_sb[1][:], d_XT.ap()[128:256, :])
            wqk_sb = [wts.tile([128, 128], bf16, tag=f"wqk{c}", name=f"wqk{c}") for c in range(2)]
            wkq_sb = [wts.tile([128, 128], bf16, tag=f"wkq{c}", name=f"wkq{c}") for c in range(2)]
            for c in range(2):
                nc.scalar.dma_start(wqk_sb[c][:], d_Wqk.ap()[c * 128:(c + 1) * 128, :])
                nc.gpsimd.dma_start(wkq_sb[c][:], d_Wkq.ap()[c * 128:(c + 1) * 128, :])
            vchunks = [(0, 128), (128, 128), (256, 1)]
            wv_sb = [wts.tile([p, DV], bf16, tag=f"wv{i}", name=f"wv{i}") for i, (v0, p) in enumerate(vchunks)]
            zt_sb = [wts.tile([p, N], bf16, tag=f"zt{i}", name=f"zt{i}") for i, (v0, p) in enumerate(vchunks)]
            for i, (v0, p) in enumerate(vchunks):
                nc.gpsimd.dma_start(wv_sb[i][:], d_Wv8.ap()[v0:v0 + p, :])
                nc.scalar.dma_start(zt_sb[i][:], d_ZT.ap()[v0:v0 + p, :])

            # ---- projections: qkT = [q;k], kqT = [k;q] along partitions ----
            # qkT rows 0:64 = qT, rows 64:128 = kT (kqT swapped).  The score
            # matmul pair for (mt even, mt odd) then runs in PE row groups
            # 0-63 / 64-127 concurrently with no on-chip duplication.
            # ---- projections, two 512-col chains per 2-bank tile, one wide
            # evac each ----
            qkT = wts.tile([128, N], bf16, tag="qkT", name="qkT")
            kqT = wts.tile([128, N], bf16, tag="kqT", name="kqT")
            for si, (dst, w_sb) in enumerate(((qkT, wqk_sb), (kqT, wkq_sb))):
                for g in range(N // SW):
                    ps = psS.tile([128, SW], f32, tag="psS", name=f"psa{si}_{g}")
                    for h in range(2):
                        for c in range(2):
                            nc.tensor.matmul(
                                ps[:, h * PW:(h + 1) * PW],
                                w_sb[c][:],
                                xt_sb[c][:, g * SW + h * PW:g * SW + (h + 1) * PW],
                                start=(c == 0),
                                stop=(c == 1),
                            )
                    if g % 2 == 0:
                        nc.vector.tensor_copy(dst[:, g * SW:(g + 1) * SW], ps[:])
                    else:
                        nc.scalar.activation(dst[:, g * SW:(g + 1) * SW], ps[:], Act.Copy)

            # ---- ZW = Z @ (Wv/sqrt(dk)); two m-tiles share one 2-bank tile
            # (each 257-col chain stays inside its own bank) ----
            zw_sb = {}

            def emit_zw_pair(mt0):
                ps = psS.tile([128, SW], f32, tag="psS", name=f"pzw{mt0}")
                for j, mt in enumerate((mt0, mt0 + 1)):
                    for i in range(3):
                        nc.tensor.matmul(
                            ps[:, j * PW:j * PW + DV],
                            zt_sb[i][:, mt * 128:(mt + 1) * 128],
                            wv_sb[i][:],
                            start=(i == 0),
                            stop=(i == 2),
                        )
                for j, mt in enumerate((mt0, mt0 + 1)):
                    zw = wts.tile([128, DV], bf16, tag=f"zw{mt}", name=f"zw{mt}")
                    if mt % 2 == 0:
                        nc.scalar.activation(zw[:], ps[:, j * PW:j * PW + DV], Act.Copy)
                    else:
                        nc.vector.tensor_copy(zw[:], ps[:, j * PW:j * PW + DV])
                    zw_sb[mt] = zw

            # ---- score pair: masked[mt] = relu(sT)*maskT for mts (2pr,2pr+1) ----
            # Elementwise runs on 1024-wide units; per pair there are 4 units.
            # pattern: tuple of unit modes, "v" = DVE fused max*mult,
            # "g"/"d" = ACT relu then GpSimd/DVE bf16 multiply.
            masked_sb = {}
            warm_ps = []  # recently consumed score tiles for keep-warm matmuls

            def emit_score_pair(pr, pattern):
                mts = (2 * pr, 2 * pr + 1)
                mds, mks = [], []
                for mt in mts:
                    mk = maskp.tile([128, N], bf16, tag="mask", name=f"mk{mt}")
                    if mt == 0:
                        # transitive gate: the whole mask stream (sync queue,
                        # FIFO) starts only after X has landed, so the
                        # projection inputs get the DMA bandwidth first.
                        nc.sync.dma_start(mk[0:1, 0:2], xt_sb[0][0:1, 0:2])
                        nc.sync.dma_start(mk[0:1, 2:4], xt_sb[1][0:1, 0:2])
                    nc.sync.dma_start(mk[:], d_maskT.ap()[mt * 128:(mt + 1) * 128, :])
                    mks.append(mk)
                    md = mskdp.tile([128, N], bf16, tag="masked", name=f"md{mt}")
                    mds.append(md)
                    masked_sb[mt] = md
                for u in range(N // SW):
                    pss = []
                    for j, mt in enumerate(mts):
                        ro = DK * j
                        lhs_t = (kqT, qkT)[j]  # rows ro:ro+64 hold kT
                        rhs_t = (qkT, kqT)[j]  # rows ro:ro+64 hold qT
                        ps = psS.tile([128, SW], f32, tag="psS", name=f"pss{mt}_{u}")
                        for h in range(SW // PW):
                            c0 = u * SW + h * PW
                            nc.tensor.matmul(
                                ps[:, h * PW:(h + 1) * PW],
                                lhs_t[ro:ro + DK, mt * 128:(mt + 1) * 128],
                                rhs_t[ro:ro + DK, c0:c0 + PW],
                                start=True,
                                stop=True,
                            )
                        pss.append(ps)
                    for j, mt in enumerate(mts):
                        sl = slice(u * SW, (u + 1) * SW)
                        mode = pattern[2 * u + j]
                        if mode == "v":
                            nc.vector.scalar_tensor_tensor(
                                mds[j][:, sl], pss[j][:], 0.0, mks[j][:, sl],
                                Alu.max, Alu.mult,
                            )
                        else:
                            rl = rlp.tile([128, SW], bf16, tag="rl", name=f"rl{mt}_{u}")
                            nc.scalar.activation(rl[:], pss[j][:], Act.Relu)
                            eng = nc.gpsimd if mode == "g" else nc.vector
                            eng.tensor_mul(mds[j][:, sl], rl[:], mks[j][:, sl])
                        warm_ps.append(pss[j])

            def emit_c_pair(nt0, mt_range, to_pA=False):
                # two 8-matmul chains (nt0, nt0+1) into the two banks of one
                # 2-bank tile, then two narrow evacs (pA spill or add+store)
                ps = psS.tile([128, SW], f32, tag="psS", name=f"psc{nt0}_{mt_range[0]}")
                for j, nt in enumerate((nt0, nt0 + 1)):
                    for i, mt in enumerate(mt_range):
                        nc.tensor.matmul(
                            ps[:, j * PW:j * PW + DV],
                            masked_sb[mt][:, nt * 128:(nt + 1) * 128],
                            zw_sb[mt][:],
                            start=(i == 0),
                            stop=(i == len(mt_range) - 1),
                        )
                if to_pA:
                    for j, nt in enumerate((nt0, nt0 + 1)):
                        pa = pAp.tile([128, DV], f32, tag="pA", name=f"pa{nt}")
                        partials[nt] = pa
                        psl = ps[:, j * PW:j * PW + DV]
                        if nt % 4 == 3:
                            nc.vector.tensor_copy(pa[:], psl)
                        else:
                            nc.scalar.activation(pa[:], psl, Act.Copy)
                    return
                ot = outp.tile([128, 2 * DV], f32, tag="out", name=f"ot{nt0}")
                for j, nt in enumerate((nt0, nt0 + 1)):
                    psl = ps[:, j * PW:j * PW + DV]
                    nc.vector.tensor_add(ot[:, j * DV:(j + 1) * DV], psl, partials[nt][:])
                nc.sync.dma_start(
                    d_out.ap()[:, nt0 * DV:(nt0 + 2) * DV], ot[:]
                )

            # phase A: zw first (dedicated early slice of the psS ring), then
            # scores for m rows 0..1023 (mask DMA-paced).
            PAT_A = ("v", "g", "v", "d")
            PAT_B = ("v", "g", "d", "d")
            PAT_TAIL = ("v", "d", "v", "g")
            for mt0 in range(0, NT, 2):
                emit_zw_pair(mt0)
            for pr in range(4):
                emit_score_pair(pr, PAT_A)

            # phase B: C half-A (mt 0..7) interleaved with scores for rows
            # 1024..2047 so the PE never waits on mask DMA.
            partials = {}
            half_a = list(range(NT // 2))
            for k in range(NT // 2):
                if k % 2 == 0 and 4 + k // 2 < 8:
                    pr = 4 + k // 2
                    emit_score_pair(pr, PAT_TAIL if pr >= 6 else PAT_B)
                emit_c_pair(2 * k, half_a, to_pA=True)

            # phase C: C half-B (mt 8..15) + partial add + store, chains
            # rotated so they start as their masked rows land.
            for k in range(NT // 2):
                r = (2 * k) % 8
                rot = [NT // 2 + ((r + i) % 8) for i in range(NT // 2)]
                emit_c_pair(2 * k, rot)

    return nc


def kernel(Z_l, X_l, M_mask, Wq, Wk, Wv):
    global LAST_EXEC_NS
    _apply_bir_patch()

    trace = os.environ.get("KERNEL_TRACE", "0") == "1"
    if trace:
        _install_profile_shim()

    from concourse.bass_utils import run_bass_kernel_spmd

    Z_l = np.asarray(Z_l, dtype=np.float32)
    X_l = np.asarray(X_l, dtype=np.float32)
    M_mask = np.asarray(M_mask, dtype=np.float32)
    Wq = np.asarray(Wq, dtype=np.float32)
    Wk = np.asarray(Wk, dtype=np.float32)
    Wv = np.asarray(Wv, dtype=np.float32)

    import ml_dtypes
    bf = ml_dtypes.bfloat16

    # Host-side layout prep (transpose + bf16 cast) + scale fold.
    XT = np.ascontiguousarray(X_l.transpose(0, 2, 1)).astype(bf)     # [B, D, N]
    ZT = np.ascontiguousarray(Z_l.transpose(0, 2, 1)).astype(bf)     # [B, DV, N]
    MT = np.ascontiguousarray(M_mask.transpose(0, 2, 1)).astype(bf)  # [B, N(m), N(n)]
    Wv8 = (Wv / np.sqrt(np.float32(DK))).astype(bf)
    Wqk = np.ascontiguousarray(np.concatenate([Wq, Wk], axis=1)).astype(bf)  # [D, 128]
    Wkq = np.ascontiguousarray(np.concatenate([Wk, Wq], axis=1)).astype(bf)  # [D, 128]

    if "nc" not in _CACHE:
        _CACHE["nc"] = _build_nc()
    nc = _CACHE["nc"]

    in_maps = [
        {
            "maskT": MT[b],
            "XT": XT[b],
            "ZT": ZT[b],
            "Wqk": Wqk,
            "Wkq": Wkq,
            "Wv8": Wv8,
        }
        for b in range(B)
    ]
    try:
        res = run_bass_kernel_spmd(nc, in_maps, core_ids=list(range(B)), trace=trace)
    except Exception:
        # A prior (profiled) run can leave an execution unit wedged; the failed
        # attempt clears it and a retry goes through.
        res = run_bass_kernel_spmd(nc, in_maps, core_ids=list(range(B)), trace=trace)
    _CACHE["last_res"] = res
    if trace:
        LAST_EXEC_NS = res.exec_time_ns
    out = np.stack(
        [
            res.results[b]["out"].reshape(128, NT, DV).transpose(1, 0, 2).reshape(N, DV)
            for b in range(B)
        ],
        axis=0,
    )
    return out

